# revision 6
# baseline (speedup 1.0000x reference)
"""Trainium2 Bass kernel: single-head cross-attention (B=8, L=2048, D=768).

Sharding: one batch element per NeuronCore (8-way data parallel, no
collectives).

Per-core layout strategy (all matmuls N=512, float32r operands = TF32-like
precision at full PE rate):
  - Host uploads video/audio pre-transposed (xvT/xaT: [D, L]) and weights
    pre-transposed (wT: [D, D]), so every contraction has its reduction
    axis on SBUF partitions with zero on-device transposes of inputs.
  - Q^T[e,q] and K^T[e,k] are built weight-stationary; V[k,e] is built
    audio-stationary (natural layout), with Wv padded to 1024 cols so both
    of its matmuls are N=512. Q^T spills to DRAM scratch and streams back
    one q-block at a time (K^T and V stay resident in SBUF).
  - Attention runs transposed: scores^T[k,q] = K^T_tile.T @ Q^T, exp via
    ScalarE (scale folded in; no max-subtraction needed since scores are
    ~N(0,1) and bounded), denominator via a ones-row matmul, and
    O^T[e,q] += V_tile.T @ exp^T accumulated over k in PSUM.
  - The AV/den matmuls for k-chunk t-1 are emitted after the scores
    matmuls of chunk t, so the PE never stalls on the exp activation.
  - Output tiles are transposed back (PE transpose-mode), multiplied by
    the reciprocal denominator, and DMA'd out in natural [q, e] layout.
"""

import numpy as np

B, L, D = 8, 2048, 768
EC = D // 128          # 6 contraction / e-tiles
KC = L // 128          # 16 k-tiles
QB = 512               # q block (PSUM bank width in fp32)
NQB = L // QB          # 4
NJ = L // 512          # moving-dim chunks for projections
SCALE = 1.0 / float(np.sqrt(D))

_CACHE = {}


def _build_nc():
    import concourse.mybir as mybir
    import concourse.tile as tile
    from concourse import bacc

    F32 = mybir.dt.float32
    F32R = mybir.dt.float32r
    EXP = mybir.ActivationFunctionType.Exp
    ADD = mybir.AluOpType.add
    MULT = mybir.AluOpType.mult

    nc = bacc.Bacc("TRN2", debug=False)
    xvt = nc.dram_tensor("xvt", [D, L], F32R, kind="ExternalInput").ap()
    xat = nc.dram_tensor("xat", [D, L], F32R, kind="ExternalInput").ap()
    wqt = nc.dram_tensor("wqt", [D, D], F32R, kind="ExternalInput").ap()
    wkt = nc.dram_tensor("wkt", [D, D], F32R, kind="ExternalInput").ap()
    wvt = nc.dram_tensor("wvt", [D, 1024], F32R, kind="ExternalInput").ap()
    bqc = nc.dram_tensor("bqc", [128, EC], F32, kind="ExternalInput").ap()
    bkc = nc.dram_tensor("bkc", [128, EC], F32, kind="ExternalInput").ap()
    bvb = nc.dram_tensor("bvb", [128, D], F32, kind="ExternalInput").ap()
    idin = nc.dram_tensor("idin", [128, 128], F32, kind="ExternalInput").ap()
    onin = nc.dram_tensor("onin", [128, 1], F32R, kind="ExternalInput").ap()
    out = nc.dram_tensor("out", [L, D], F32, kind="ExternalOutput").ap()
    qtd = nc.dram_tensor("qtd", [EC, 128, L], F32R).ap()  # Q^T spill scratch

    with tile.TileContext(nc) as tc:
        with tc.tile_pool(name="persist", bufs=1) as pp, \
             tc.tile_pool(name="wstream", bufs=12) as wst:
            kt = pp.tile([128, EC, L], F32R, tag="kt")
            v = pp.tile([128, KC, D], F32R, tag="v")
            bq_sb = pp.tile([128, EC], F32, tag="bq")
            bk_sb = pp.tile([128, EC], F32, tag="bk")
            bvb_sb = pp.tile([128, D], F32, tag="bvb")
            id_sb = pp.tile([128, 128], F32, tag="ident")
            ones_sb = pp.tile([128, 1], F32R, tag="ones")
            nc.sync.dma_start(out=bq_sb, in_=bqc)
            nc.sync.dma_start(out=bk_sb, in_=bkc)
            nc.sync.dma_start(out=bvb_sb, in_=bvb)
            nc.sync.dma_start(out=id_sb, in_=idin)
            nc.sync.dma_start(out=ones_sb, in_=onin)

            def proj_mms(p, x_sb, w_dram, e):
                # p[128e, L] += W[e,:] @ x^T (weight chunks stationary)
                for c in range(EC):
                    w = wst.tile([128, 128], F32R, tag="wch")
                    nc.sync.dma_start(
                        out=w,
                        in_=w_dram[c * 128:(c + 1) * 128, e * 128:(e + 1) * 128])
                    for j in range(NJ):
                        nc.tensor.matmul(
                            p[:, j * 512:(j + 1) * 512], w,
                            x_sb[:, c, j * 512:(j + 1) * 512],
                            start=(c == 0), stop=(c == EC - 1))

            # ---- Phase 1: Q^T from video -> DRAM scratch ----
            with tc.tile_pool(name="xv", bufs=1) as xvp, \
                 tc.tile_pool(name="stage", bufs=3) as stg, \
                 tc.tile_pool(name="ps1", bufs=2, space="PSUM") as ps1:
                xv_sb = xvp.tile([128, EC, L], F32R, tag="xvt")
                for c in range(EC):
                    nc.sync.dma_start(out=xv_sb[:, c, :],
                                      in_=xvt[c * 128:(c + 1) * 128, :])
                for e in range(EC):
                    p = ps1.tile([128, L], mybir.dt.float32, tag="pq")
                    proj_mms(p, xv_sb, wqt, e)
                    qs = stg.tile([128, L], F32R, tag="qstage")
                    nc.vector.tensor_scalar_add(qs, p, bq_sb[:, e:e + 1])
                    nc.sync.dma_start(out=qtd[e], in_=qs)

            # prefetch attention's first q-block while audio phases run
            qbp_cm = tc.tile_pool(name="qblk", bufs=2)
            qbp = qbp_cm.__enter__()
            qtb0 = qbp.tile([128, EC, QB], F32R, tag="qtb", name="qtb_pre")
            for c in range(EC):
                nc.sync.dma_start(out=qtb0[:, c, :], in_=qtd[c, :, 0:QB])

            # ---- Phases 2+3: V then K^T from audio ----
            with tc.tile_pool(name="xa", bufs=1) as xap:
                with tc.tile_pool(name="wv", bufs=1) as wvp, \
                     tc.tile_pool(name="ps2", bufs=2, space="PSUM") as ps2:
                    xa_sb = xap.tile([128, EC, L], F32R, tag="xat")
                    wv_sb = wvp.tile([128, EC, 1024], F32R, tag="wv")
                    for c in range(EC):
                        nc.sync.dma_start(out=xa_sb[:, c, :],
                                          in_=xat[c * 128:(c + 1) * 128, :])
                        nc.sync.dma_start(out=wv_sb[:, c, :],
                                          in_=wvt[c * 128:(c + 1) * 128, :])
                    for k in range(KC):
                        p = ps2.tile([128, 1024], mybir.dt.float32, tag="pv")
                        for c in range(EC):
                            for j in range(2):
                                nc.tensor.matmul(
                                    p[:, j * 512:(j + 1) * 512],
                                    xa_sb[:, c, k * 128:(k + 1) * 128],
                                    wv_sb[:, c, j * 512:(j + 1) * 512],
                                    start=(c == 0), stop=(c == EC - 1))
                        nc.vector.tensor_tensor(v[:, k, :], p[:, 0:D], bvb_sb, ADD)
                with tc.tile_pool(name="ps3", bufs=2, space="PSUM") as ps3:
                    for e in range(EC):
                        p = ps3.tile([128, L], mybir.dt.float32, tag="pk")
                        proj_mms(p, xa_sb, wkt, e)
                        nc.vector.tensor_scalar_add(kt[:, e, :], p,
                                                    bk_sb[:, e:e + 1])

            # ---- Phase 4: attention ----
            with tc.tile_pool(name="aux", bufs=3) as aux, \
                 tc.tile_pool(name="outp", bufs=5) as outp, \
                 tc.tile_pool(name="ps_sc", bufs=1, space="PSUM") as ps_sc, \
                 tc.tile_pool(name="ps_den", bufs=1, space="PSUM") as ps_den, \
                 tc.tile_pool(name="ps_ot", bufs=6, space="PSUM") as ps_ot:
                for qb in range(NQB):
                    q0 = qb * QB
                    if qb == 0:
                        qtb = qtb0
                    else:
                        qtb = qbp.tile([128, EC, QB], F32R, tag="qtb",
                                       name=f"qtb{qb}")
                        for c in range(EC):
                            nc.sync.dma_start(out=qtb[:, c, :],
                                              in_=qtd[c, :, q0:q0 + QB])
                    den_p = ps_den.tile([1, QB], mybir.dt.float32, tag="den")
                    ot_p = [ps_ot.tile([128, QB], mybir.dt.float32, tag="ot",
                                       name=f"ot{qb}_{e}") for e in range(EC)]
                    exps = [None] * KC

                    def emit_av(k):
                        ex = exps[k]
                        nc.tensor.matmul(den_p, ones_sb, ex,
                                         start=(k == 0), stop=(k == KC - 1))
                        for e in range(EC):
                            nc.tensor.matmul(
                                ot_p[e], v[:, k, e * 128:(e + 1) * 128], ex,
                                start=(k == 0), stop=(k == KC - 1))

                    for k in range(KC):
                        sc = ps_sc.tile([128, QB], mybir.dt.float32, tag="sc")
                        for c in range(EC):
                            nc.tensor.matmul(
                                sc, kt[:, c, k * 128:(k + 1) * 128],
                                qtb[:, c, :],
                                start=(c == 0), stop=(c == EC - 1))
                        ex = aux.tile([128, QB], F32R, tag="exp")
                        nc.scalar.activation(ex, sc, EXP, scale=SCALE)
                        exps[k] = ex
                        if k > 0:
                            emit_av(k - 1)
                    emit_av(KC - 1)

                    # tail: copy O^T out of PSUM (frees ot slots), invert the
                    # denominator, transpose back, scale, store.
                    osbs = []
                    for e in range(EC):
                        osb = aux.tile([128, QB], F32, tag="otsb",
                                       name=f"osb{qb}_{e}")
                        nc.vector.tensor_copy(osb, ot_p[e])
                        osbs.append(osb)
                    den_sb = aux.tile([1, QB], F32, tag="densb")
                    nc.vector.tensor_copy(den_sb, den_p)
                    dent_p = ps_den.tile([128, NQB], mybir.dt.float32,
                                         tag="den", name=f"dent{qb}")
                    for i in range(QB // 128):
                        nc.tensor.transpose(dent_p[:, i:i + 1],
                                            den_sb[0:1, i * 128:(i + 1) * 128],
                                            id_sb[0:1, 0:1])
                    dent_sb = aux.tile([128, NQB], F32, tag="dentsb")
                    nc.vector.tensor_copy(dent_sb, dent_p)
                    rec = aux.tile([128, NQB], F32, tag="rec")
                    nc.vector.reciprocal(rec, dent_sb)
                    out_sbs = [outp.tile([128, D], F32, tag="outsb",
                                         name=f"outsb{qb}_{i}")
                               for i in range(QB // 128)]
                    for e in range(EC):
                        for i in range(QB // 128):
                            tp = ps_ot.tile([128, 128], mybir.dt.float32,
                                            tag="ot", name=f"tp{qb}_{e}_{i}")
                            nc.tensor.transpose(
                                tp, osbs[e][:, i * 128:(i + 1) * 128], id_sb)
                            nc.vector.tensor_scalar(
                                out_sbs[i][:, e * 128:(e + 1) * 128], tp,
                                rec[:, i:i + 1], None, MULT)
                    for i in range(QB // 128):
                        nc.sync.dma_start(
                            out=out[q0 + i * 128:q0 + (i + 1) * 128, :],
                            in_=out_sbs[i])
            qbp_cm.__exit__(None, None, None)
    nc.compile()
    return nc


def _get_nc():
    if "nc" not in _CACHE:
        _CACHE["nc"] = _build_nc()
    return _CACHE["nc"]


def _prep_in_maps(video_features, audio_features, Wq, bq, Wk, bk, Wv, bv):
    f32 = np.float32
    wqt = np.ascontiguousarray(np.asarray(Wq, f32).T)
    wkt = np.ascontiguousarray(np.asarray(Wk, f32).T)
    wvt = np.zeros((D, 1024), f32)
    wvt[:, :D] = np.asarray(Wv, f32).T
    bqc = np.ascontiguousarray(np.asarray(bq, f32).reshape(EC, 128).T)
    bkc = np.ascontiguousarray(np.asarray(bk, f32).reshape(EC, 128).T)
    bvb = np.ascontiguousarray(np.broadcast_to(np.asarray(bv, f32), (128, D)))
    idin = np.eye(128, dtype=f32)
    onin = np.ones((128, 1), f32)
    shared = dict(wqt=wqt, wkt=wkt, wvt=wvt, bqc=bqc, bkc=bkc, bvb=bvb,
                  idin=idin, onin=onin)
    in_maps = []
    for b in range(B):
        xvt = np.ascontiguousarray(np.asarray(video_features[b], f32).T)
        xat = np.ascontiguousarray(np.asarray(audio_features[b], f32).T)
        in_maps.append(dict(xvt=xvt, xat=xat, **shared))
    return in_maps


def run_on_hw(inputs, trace=False, trace_cores=None):
    from concourse.bass_utils import run_bass_kernel_spmd
    nc = _get_nc()
    in_maps = _prep_in_maps(**inputs)
    r = run_bass_kernel_spmd(nc, in_maps, list(range(B)), trace=trace,
                             trace_cores=trace_cores)
    out = np.stack([r.results[i]["out"] for i in range(B)]).astype(np.float32)
    return out, r


def kernel(**inputs):
    out, _ = run_on_hw(inputs, trace=False)
    return out


# revision 7
# speedup vs baseline: 1.0447x; 1.0447x over previous
"""Trainium2 Bass kernel: single-head cross-attention (B=8, L=2048, D=768).

Sharding: one batch element per NeuronCore (8-way data parallel, no
collectives).

Per-core layout strategy (all matmuls N=512, float32r operands = TF32-like
precision at full PE rate):
  - Host uploads video/audio pre-transposed (xvT/xaT: [D, L]) and weights
    pre-transposed (wT: [D, D]), so every contraction has its reduction
    axis on SBUF partitions with zero on-device transposes of inputs.
  - Inputs stream through SBUF in [128, 6, 512] j-blocks (12 KB/partition)
    so compute starts ~3us after launch and DMA overlaps compute; weights
    are resident per phase. Q^T, K^T and V all stay resident in SBUF.
  - Q^T[e,q] and K^T[e,k] are built weight-stationary; V[k,e] is built
    audio-stationary (natural layout), with Wv padded to 1024 cols so both
    of its matmuls are N=512.
  - Attention runs transposed: scores^T[k,q] = K^T_tile.T @ Q^T, exp via
    ScalarE (scale folded in; no max-subtraction needed since scores are
    ~N(0,1) and bounded), denominator via a ones-row matmul, and
    O^T[e,q] += V_tile.T @ exp^T accumulated over k in PSUM.
  - The AV/den matmuls for k-chunk t-1 are emitted after the scores
    matmuls of chunk t, so the PE never stalls on the exp activation.
  - Output tiles are transposed back (PE transpose-mode), multiplied by
    the reciprocal denominator, and DMA'd out in natural [q, e] layout.
"""

import numpy as np

B, L, D = 8, 2048, 768
EC = D // 128          # 6 contraction / e-tiles
KC = L // 128          # 16 k-tiles
QB = 512               # q block (PSUM bank width in fp32)
NQB = L // QB          # 4
NJ = L // 512          # j-blocks over L
SCALE = 1.0 / float(np.sqrt(D))

_CACHE = {}


def _build_nc():
    import concourse.mybir as mybir
    import concourse.tile as tile
    from concourse import bacc

    F32 = mybir.dt.float32
    F32R = mybir.dt.float32r
    EXP = mybir.ActivationFunctionType.Exp
    ADD = mybir.AluOpType.add
    MULT = mybir.AluOpType.mult

    nc = bacc.Bacc("TRN2", debug=False)
    xvt = nc.dram_tensor("xvt", [D, L], F32R, kind="ExternalInput").ap()
    xat = nc.dram_tensor("xat", [D, L], F32R, kind="ExternalInput").ap()
    wqt = nc.dram_tensor("wqt", [D, D], F32R, kind="ExternalInput").ap()
    wkt = nc.dram_tensor("wkt", [D, D], F32R, kind="ExternalInput").ap()
    wvt = nc.dram_tensor("wvt", [D, 1024], F32R, kind="ExternalInput").ap()
    bqc = nc.dram_tensor("bqc", [128, EC], F32, kind="ExternalInput").ap()
    bkc = nc.dram_tensor("bkc", [128, EC], F32, kind="ExternalInput").ap()
    bvb = nc.dram_tensor("bvb", [128, D], F32, kind="ExternalInput").ap()
    idin = nc.dram_tensor("idin", [128, 128], F32, kind="ExternalInput").ap()
    onin = nc.dram_tensor("onin", [128, 1], F32R, kind="ExternalInput").ap()
    out = nc.dram_tensor("out", [L, D], F32, kind="ExternalOutput").ap()

    with tile.TileContext(nc) as tc:
        with tc.tile_pool(name="persist", bufs=1) as pp:
            qt = pp.tile([128, EC, L], F32R, tag="qt")
            kt = pp.tile([128, EC, L], F32R, tag="kt")
            v = pp.tile([128, KC, D], F32R, tag="v")
            bq_sb = pp.tile([128, EC], F32, tag="bq")
            bk_sb = pp.tile([128, EC], F32, tag="bk")
            bvb_sb = pp.tile([128, D], F32, tag="bvb")
            id_sb = pp.tile([128, 128], F32, tag="ident")
            ones_sb = pp.tile([128, 1], F32R, tag="ones")
            nc.sync.dma_start(out=bq_sb, in_=bqc)
            nc.sync.dma_start(out=bk_sb, in_=bkc)
            nc.sync.dma_start(out=bvb_sb, in_=bvb)
            nc.sync.dma_start(out=id_sb, in_=idin)
            nc.sync.dma_start(out=ones_sb, in_=onin)

            def load_w(pool, w_dram, width, tag):
                w_sb = pool.tile([128, EC, width], F32R, tag=tag)
                for c in range(EC):
                    nc.sync.dma_start(out=w_sb[:, c, :],
                                      in_=w_dram[c * 128:(c + 1) * 128, :])
                return w_sb

            def proj_pass(dst, x_dram, w_sb, bias_sb, jpool, ppool, pfx):
                # dst[:, e, jb] = W[e-tile, :] @ x^T[:, jb]  (+bias), streamed
                # over j-blocks of 512 with weights resident.
                for jb in range(NJ):
                    xj = jpool.tile([128, EC, 512], F32R, tag="xjb",
                                    name=f"x{pfx}{jb}")
                    for c in range(EC):
                        nc.sync.dma_start(
                            out=xj[:, c, :],
                            in_=x_dram[c * 128:(c + 1) * 128,
                                       jb * 512:(jb + 1) * 512])
                    for e in range(EC):
                        p = ppool.tile([128, 512], mybir.dt.float32, tag="pp",
                                       name=f"p{pfx}{jb}_{e}")
                        for c in range(EC):
                            nc.tensor.matmul(
                                p, w_sb[:, c, e * 128:(e + 1) * 128],
                                xj[:, c, :],
                                start=(c == 0), stop=(c == EC - 1))
                        nc.vector.tensor_scalar_add(
                            dst[:, e, jb * 512:(jb + 1) * 512], p,
                            bias_sb[:, e:e + 1])

            # ---- Phase 1: Q^T from video (resident in SBUF) ----
            with tc.tile_pool(name="wq", bufs=1) as wqp, \
                 tc.tile_pool(name="xj1", bufs=2) as xj1, \
                 tc.tile_pool(name="ps1", bufs=4, space="PSUM") as ps1:
                wq_sb = load_w(wqp, wqt, D, "wq")
                proj_pass(qt, xvt, wq_sb, bq_sb, xj1, ps1, "v")

            # ---- Phase 2: V from audio (audio-stationary, j-blocked) ----
            with tc.tile_pool(name="wv", bufs=1) as wvp, \
                 tc.tile_pool(name="xj2", bufs=2) as xj2, \
                 tc.tile_pool(name="ps2", bufs=2, space="PSUM") as ps2:
                wv_sb = load_w(wvp, wvt, 1024, "wv")
                for jb in range(NJ):
                    xj = xj2.tile([128, EC, 512], F32R, tag="xjb",
                                  name=f"xa{jb}")
                    for c in range(EC):
                        nc.sync.dma_start(
                            out=xj[:, c, :],
                            in_=xat[c * 128:(c + 1) * 128,
                                    jb * 512:(jb + 1) * 512])
                    for kk in range(4):          # 4 k-tiles per j-block
                        k = jb * 4 + kk
                        p = ps2.tile([128, 1024], mybir.dt.float32, tag="pv",
                                     name=f"pv{k}")
                        for c in range(EC):
                            for j in range(2):
                                nc.tensor.matmul(
                                    p[:, j * 512:(j + 1) * 512],
                                    xj[:, c, kk * 128:(kk + 1) * 128],
                                    wv_sb[:, c, j * 512:(j + 1) * 512],
                                    start=(c == 0), stop=(c == EC - 1))
                        nc.vector.tensor_tensor(v[:, k, :], p[:, 0:D],
                                                bvb_sb, ADD)

            # ---- Phase 3: K^T from audio (resident in SBUF) ----
            with tc.tile_pool(name="wk", bufs=1) as wkp, \
                 tc.tile_pool(name="xj3", bufs=2) as xj3, \
                 tc.tile_pool(name="ps3", bufs=4, space="PSUM") as ps3:
                wk_sb = load_w(wkp, wkt, D, "wk")
                proj_pass(kt, xat, wk_sb, bk_sb, xj3, ps3, "a")

            # ---- Phase 4: attention ----
            with tc.tile_pool(name="aux", bufs=3) as aux, \
                 tc.tile_pool(name="outp", bufs=5) as outp, \
                 tc.tile_pool(name="ps_sc", bufs=1, space="PSUM") as ps_sc, \
                 tc.tile_pool(name="ps_den", bufs=1, space="PSUM") as ps_den, \
                 tc.tile_pool(name="ps_ot", bufs=6, space="PSUM") as ps_ot:
                for qb in range(NQB):
                    q0 = qb * QB
                    den_p = ps_den.tile([1, QB], mybir.dt.float32, tag="den")
                    ot_p = [ps_ot.tile([128, QB], mybir.dt.float32, tag="ot",
                                       name=f"ot{qb}_{e}") for e in range(EC)]
                    exps = [None] * KC

                    def emit_av(k):
                        ex = exps[k]
                        nc.tensor.matmul(den_p, ones_sb, ex,
                                         start=(k == 0), stop=(k == KC - 1))
                        for e in range(EC):
                            nc.tensor.matmul(
                                ot_p[e], v[:, k, e * 128:(e + 1) * 128], ex,
                                start=(k == 0), stop=(k == KC - 1))

                    for k in range(KC):
                        sc = ps_sc.tile([128, QB], mybir.dt.float32, tag="sc")
                        for c in range(EC):
                            nc.tensor.matmul(
                                sc, kt[:, c, k * 128:(k + 1) * 128],
                                qt[:, c, q0:q0 + QB],
                                start=(c == 0), stop=(c == EC - 1))
                        ex = aux.tile([128, QB], F32R, tag="exp")
                        nc.scalar.activation(ex, sc, EXP, scale=SCALE)
                        exps[k] = ex
                        if k > 0:
                            emit_av(k - 1)
                    emit_av(KC - 1)

                    # tail: copy O^T out of PSUM (frees ot slots), invert the
                    # denominator, transpose back, scale, store.
                    osbs = []
                    for e in range(EC):
                        osb = aux.tile([128, QB], F32, tag="otsb",
                                       name=f"osb{qb}_{e}")
                        nc.vector.tensor_copy(osb, ot_p[e])
                        osbs.append(osb)
                    den_sb = aux.tile([1, QB], F32, tag="densb")
                    nc.vector.tensor_copy(den_sb, den_p)
                    dent_p = ps_den.tile([128, NQB], mybir.dt.float32,
                                         tag="den", name=f"dent{qb}")
                    for i in range(QB // 128):
                        nc.tensor.transpose(dent_p[:, i:i + 1],
                                            den_sb[0:1, i * 128:(i + 1) * 128],
                                            id_sb[0:1, 0:1])
                    dent_sb = aux.tile([128, NQB], F32, tag="dentsb")
                    nc.vector.tensor_copy(dent_sb, dent_p)
                    rec = aux.tile([128, NQB], F32, tag="rec")
                    nc.vector.reciprocal(rec, dent_sb)
                    out_sbs = [outp.tile([128, D], F32, tag="outsb",
                                         name=f"outsb{qb}_{i}")
                               for i in range(QB // 128)]
                    for e in range(EC):
                        for i in range(QB // 128):
                            tp = ps_ot.tile([128, 128], mybir.dt.float32,
                                            tag="ot", name=f"tp{qb}_{e}_{i}")
                            nc.tensor.transpose(
                                tp, osbs[e][:, i * 128:(i + 1) * 128], id_sb)
                            nc.vector.tensor_scalar(
                                out_sbs[i][:, e * 128:(e + 1) * 128], tp,
                                rec[:, i:i + 1], None, MULT)
                    for i in range(QB // 128):
                        nc.sync.dma_start(
                            out=out[q0 + i * 128:q0 + (i + 1) * 128, :],
                            in_=out_sbs[i])
    nc.compile()
    return nc


def _get_nc():
    if "nc" not in _CACHE:
        _CACHE["nc"] = _build_nc()
    return _CACHE["nc"]


def _prep_in_maps(video_features, audio_features, Wq, bq, Wk, bk, Wv, bv):
    f32 = np.float32
    wqt = np.ascontiguousarray(np.asarray(Wq, f32).T)
    wkt = np.ascontiguousarray(np.asarray(Wk, f32).T)
    wvt = np.zeros((D, 1024), f32)
    wvt[:, :D] = np.asarray(Wv, f32).T
    bqc = np.ascontiguousarray(np.asarray(bq, f32).reshape(EC, 128).T)
    bkc = np.ascontiguousarray(np.asarray(bk, f32).reshape(EC, 128).T)
    bvb = np.ascontiguousarray(np.broadcast_to(np.asarray(bv, f32), (128, D)))
    idin = np.eye(128, dtype=f32)
    onin = np.ones((128, 1), f32)
    shared = dict(wqt=wqt, wkt=wkt, wvt=wvt, bqc=bqc, bkc=bkc, bvb=bvb,
                  idin=idin, onin=onin)
    in_maps = []
    for b in range(B):
        xvt = np.ascontiguousarray(np.asarray(video_features[b], f32).T)
        xat = np.ascontiguousarray(np.asarray(audio_features[b], f32).T)
        in_maps.append(dict(xvt=xvt, xat=xat, **shared))
    return in_maps


def run_on_hw(inputs, trace=False, trace_cores=None):
    from concourse.bass_utils import run_bass_kernel_spmd
    nc = _get_nc()
    in_maps = _prep_in_maps(**inputs)
    r = run_bass_kernel_spmd(nc, in_maps, list(range(B)), trace=trace,
                             trace_cores=trace_cores)
    out = np.stack([r.results[i]["out"] for i in range(B)]).astype(np.float32)
    return out, r


def kernel(**inputs):
    out, _ = run_on_hw(inputs, trace=False)
    return out
